# revision 18
# baseline (speedup 1.0000x reference)
"""DiffAttention (nn_DiffAttention) — Trainium2 Bass kernel, 8 NeuronCores.

Sharding: 4 batches x 6 effective heads = 24 units -> core c gets batch
c//2 and effective heads [3*(c%2), 3*(c%2)+3).  Each core computes its
q/k/v projections (column-sliced), both softmax maps per eff-head, the
differential combination, head RMS-norm, and its partial output
projection; the host sums the two per-batch partials and adds bo.

All matmuls run in bf16 (fp32 PSUM accumulation).  Softmax is computed
without max-subtraction (scores are O(5) here) and without explicit
normalization: the softmax denominators d1, d2 enter through
lamp = lambda*d1/d2 and the eps-correction of the scale-invariant
RMS norm (rms(c*u) ~ c*rms(u)):

  E1 = exp(S1), E2 = exp(S2)         (ACT, accum_out -> d1, d2)
  diffn = lamp*E2 - E1               (DVE)
  u = diffn @ V                      (PE; ref out_pre = -(1/d1)*u)
  r = rsqrt(mean_dv(u^2) + eps*d1^2) (DVE + ACT ln/exp)
  out_n = u*r*(-0.2*wn)              (signs/scales cancel exactly)
  y += out_n @ Wo_slice              (PE)
"""

import os
import sys
from contextlib import ExitStack

import numpy as np

try:
    import concourse.bass as bass  # noqa: F401
except ImportError:
    for _p in ("/opt/trn_rl_repo", os.path.expanduser("~/trn_rl_repo")):
        if os.path.isdir(_p):
            sys.path.insert(0, _p)
            break
    import concourse.bass as bass  # noqa: F401

import ml_dtypes
import concourse.tile as tile
from concourse import bacc, bass_utils, mybir
from concourse.bass import ts
from concourse.masks import make_identity

F32 = mybir.dt.float32
BF16 = mybir.dt.bfloat16
AF = mybir.ActivationFunctionType
OP = mybir.AluOpType

DT_MODE = os.environ.get("KDT", "pe")  # "dma" | "pe" | "dmaq" | "hybrid"

B = 4
N = 2048
D = 768
HD = 64
EH = 3  # eff heads per core
NT = N // 128
NCORES = 8
EPS = 1e-5
LAMBDA_INIT = 0.8
BF = ml_dtypes.bfloat16


def _body(ctx, tc, xT, wq, wk, wv, wo, bq, bk, bv128, lam128, y):
    nc = tc.nc
    reg_dma = nc.gpsimd if DT_MODE == "dmaq" else nc.sync

    const = ctx.enter_context(tc.tile_pool(name="const", bufs=1))
    wpool = ctx.enter_context(tc.tile_pool(name="wpool", bufs=1))
    xpool = ctx.enter_context(tc.tile_pool(name="xpool", bufs=1))
    qkv = ctx.enter_context(tc.tile_pool(name="qkv", bufs=1))
    # psum: s 2x2 banks + pv 4x1 banks = 8 banks
    spool = ctx.enter_context(tc.tile_pool(name="spool", bufs=2, space="PSUM"))
    pvp = ctx.enter_context(tc.tile_pool(name="pvp", bufs=4, space="PSUM"))
    epool = ctx.enter_context(tc.tile_pool(name="epool", bufs=6))
    dpool = ctx.enter_context(tc.tile_pool(name="dpool", bufs=4))
    dtp = ctx.enter_context(tc.tile_pool(name="dtp", bufs=2))
    small = ctx.enter_context(tc.tile_pool(name="small", bufs=4))
    onp = ctx.enter_context(tc.tile_pool(name="onp", bufs=3))
    ysb = ctx.enter_context(tc.tile_pool(name="ysb", bufs=2))

    lam_t = const.tile([128, 1], F32)
    reg_dma.dma_start(lam_t[:], lam128)
    ident = const.tile([128, 128], BF16)
    make_identity(nc, ident[:])
    bv_t = const.tile([128, 384], F32)
    reg_dma.dma_start(bv_t[:], bv128)
    bq_t = const.tile([128, 3], F32)
    bk_t = const.tile([128, 3], F32)
    for i in range(3):
        reg_dma.dma_start(bq_t[:, i : i + 1], bq[i : i + 1, :].rearrange("a b -> b a"))
        reg_dma.dma_start(bk_t[:, i : i + 1], bk[i : i + 1, :].rearrange("a b -> b a"))

    xt_t = []
    for i in range(6):
        t = xpool.tile([128, N], BF16, tag=f"xt{i}", name=f"xt{i}")
        reg_dma.dma_start(t[:], xT[ts(i, 128), :])
        xt_t.append(t)
    w_t = {}
    for name, ap in (("q", wq), ("k", wk), ("v", wv)):
        tiles = []
        for i in range(6):
            t = wpool.tile([128, 384], BF16, tag=f"w{name}{i}", name=f"w{name}{i}")
            reg_dma.dma_start(t[:], ap[ts(i, 128), :])
            tiles.append(t)
        w_t[name] = tiles
    wo_t = []
    for h in range(EH):
        t = wpool.tile([128, D], BF16, tag=f"wo{h}", name=f"wo{h}")
        reg_dma.dma_start(t[:], wo[ts(h, 128), :])
        wo_t.append(t)

    # ---- emission helpers ----------------------------------------------
    qT, kT = [None] * 3, [None] * 3
    v_t = qkv.tile([128, NT, 384], BF16, tag="v")
    outnT = []
    for h in range(EH):
        outnT.append(qkv.tile([128, N], BF16, tag=f"outnT{h}", name=f"outnT{h}"))

    def proj_qk_chunk(m, name, cc):
        bias = bq_t if name == "q" else bk_t
        dst = qT if name == "q" else kT
        out_t = dst[m]
        ps = pvp.tile([128, 512], F32, tag="pv", name="psq")
        for kblk in range(6):
            nc.tensor.matmul(
                ps[:],
                w_t[name][kblk][:, ts(m, 128)],
                xt_t[kblk][:, ts(cc, 512)],
                start=(kblk == 0),
                stop=(kblk == 5),
            )
        nc.vector.tensor_scalar(
            out=out_t[:, ts(cc, 512)],
            in0=ps[:],
            scalar1=bias[:, m : m + 1],
            scalar2=None,
            op0=OP.add,
        )

    def proj_qk(m):
        for name in ("q", "k"):
            dst = qT if name == "q" else kT
            dst[m] = qkv.tile(
                [128, N], BF16, tag=f"{name}T{m}", name=f"{name}T{m}"
            )
        for name in ("q", "k"):
            for cc in range(4):
                proj_qk_chunk(m, name, cc)

    def proj_qk_chunks(m):
        for name in ("q", "k"):
            dst = qT if name == "q" else kT
            dst[m] = qkv.tile(
                [128, N], BF16, tag=f"{name}T{m}", name=f"{name}T{m}"
            )
        return [
            (lambda m=m, name=name, cc=cc: proj_qk_chunk(m, name, cc))
            for name in ("q", "k")
            for cc in range(4)
        ]

    def proj_v_chunks():
        return [(lambda tb=tb: proj_v(tb, tb + 2)) for tb in range(0, NT, 2)]

    def proj_v(tb0, tb1):
        for tb in range(tb0, tb1):
            ps = pvp.tile([128, 512], F32, tag="pv", name="psv")
            for kblk in range(6):
                nc.tensor.matmul(
                    ps[:, 0:384],
                    xt_t[kblk][:, ts(tb, 128)],
                    w_t["v"][kblk][:],
                    start=(kblk == 0),
                    stop=(kblk == 5),
                )
            nc.vector.tensor_tensor(
                out=v_t[:, tb, :], in0=ps[:, 0:384], in1=bv_t[:], op=OP.add
            )

    def attn_block(h, g, b, dT, lamp_g, d1_g):
        t1 = g * 4 + b
        e_t = {}
        dacc = small.tile([128, 4], F32, tag="dacc", name="dacc")
        for sub in range(2):
            e = epool.tile([128, N], BF16, tag="E", name="e")
            for half in range(2):
                ps = spool.tile([128, 1024], F32, tag="s", name="ps")
                for c2 in range(2):
                    cc = half * 2 + c2
                    nc.tensor.matmul(
                        ps[:, ts(c2, 512)],
                        qT[h][ts(sub, 64), ts(t1, 128)],
                        kT[h][ts(sub, 64), ts(cc, 512)],
                        start=True,
                        stop=True,
                    )
                nc.scalar.activation(
                    e[:, ts(half, 1024)],
                    ps[:],
                    AF.Exp,
                    accum_out=dacc[:, sub * 2 + half : sub * 2 + half + 1],
                )
            e_t[sub] = e
        d1 = d1_g[:, b : b + 1]
        nc.vector.tensor_tensor(out=d1, in0=dacc[:, 0:1], in1=dacc[:, 1:2], op=OP.add)
        d2s = small.tile([128, 1], F32, tag="d2s", name="d2s")
        nc.vector.tensor_tensor(
            out=d2s[:], in0=dacc[:, 2:3], in1=dacc[:, 3:4], op=OP.add
        )
        rec = small.tile([128, 1], F32, tag="rec", name="rec")
        nc.vector.reciprocal(rec[:], d2s[:])
        nc.vector.tensor_scalar(
            out=lamp_g[:, b : b + 1],
            in0=rec[:],
            scalar1=d1,
            scalar2=lam_t[:],
            op0=OP.mult,
            op1=OP.mult,
        )
        # diff = E1 - lamp*E2   (TS 4x + TT 2x beat one un-moded STT)
        diff = dpool.tile([128, N], BF16, tag="diff", name="diff")
        nc.vector.tensor_scalar(
            out=diff[:],
            in0=e_t[1][:],
            scalar1=lamp_g[:, b : b + 1],
            scalar2=None,
            op0=OP.mult,
        )
        diff_eng = nc.gpsimd if (g * 4 + b) % 2 == 1 else nc.vector
        diff_eng.tensor_tensor(out=diff[:], in0=e_t[0][:], in1=diff[:], op=OP.subtract)
        if DT_MODE in ("dma", "dmaq"):
            for j in range(NT):
                nc.sync.dma_start(
                    dT[:, j, ts(b, 128)], diff[:, ts(j, 128)], transpose=True
                )
        elif DT_MODE == "hybrid":
            for j in range(8, NT):
                nc.sync.dma_start(
                    dT[:, j, ts(b, 128)], diff[:, ts(j, 128)], transpose=True
                )
            for jb in range(2):
                tr = pvp.tile([128, 4, 128], BF16, tag="pv", name="trp")
                for jj in range(4):
                    nc.tensor.transpose(
                        tr[:, jj, :], diff[:, ts(jb * 4 + jj, 128)], ident[:]
                    )
                nc.vector.tensor_copy(dT[:, jb * 4 : jb * 4 + 4, ts(b, 128)], tr[:])
        else:
            for jb in range(4):
                tr = pvp.tile([128, 4, 128], BF16, tag="pv", name="trp")
                for jj in range(4):
                    nc.tensor.transpose(
                        tr[:, jj, :], diff[:, ts(jb * 4 + jj, 128)], ident[:]
                    )
                nc.vector.tensor_copy(dT[:, jb * 4 : jb * 4 + 4, ts(b, 128)], tr[:])

    def attn_group_tail(h, g, dT, lamp_g, d1_g):
        pv = pvp.tile([128, 512], F32, tag="pv", name="pv")
        for j in range(NT):
            nc.tensor.matmul(
                pv[:],
                v_t[:, j, ts(h, 128)],
                dT[:, j, :],
                start=(j == 0),
                stop=(j == NT - 1),
            )
        outT = onp.tile([128, 512], BF16, tag="outT", name="outT")
        nc.vector.tensor_copy(outT[:], pv[:])
        nat_sb = onp.tile([128, 4, 128], BF16, tag="natsb", name="natsb")
        if DT_MODE in ("dma", "dmaq"):
            for b in range(4):
                nc.sync.dma_start(nat_sb[:, b, :], outT[:, ts(b, 128)], transpose=True)
        else:
            trn = pvp.tile([128, 4, 128], BF16, tag="pv", name="trn")
            for b in range(4):
                nc.tensor.transpose(trn[:, b, :], outT[:, ts(b, 128)], ident[:])
            nc.vector.tensor_copy(nat_sb[:], trn[:])
        zg = small.tile([128, 4], F32, tag="zg", name="zg")
        scr = onp.tile([128, 128], F32, tag="scr", name="scr")
        for b in range(4):
            nc.vector.tensor_tensor(
                out=scr[:], in0=nat_sb[:, b, :], in1=nat_sb[:, b, :], op=OP.mult
            )
            nc.vector.tensor_reduce(
                out=zg[:, b : b + 1], in_=scr[:], axis=mybir.AxisListType.X, op=OP.add
            )
        nc.vector.tensor_scalar(
            out=zg[:], in0=zg[:], scalar1=1.0 / 128.0, scalar2=None, op0=OP.mult
        )
        epsd = small.tile([128, 4], F32, tag="epsd", name="epsd")
        nc.vector.tensor_tensor(out=epsd[:], in0=d1_g[:], in1=d1_g[:], op=OP.mult)
        nc.vector.scalar_tensor_tensor(
            out=zg[:], in0=epsd[:], scalar=EPS, in1=zg[:], op0=OP.mult, op1=OP.add
        )
        lz = small.tile([128, 4], F32, tag="lz", name="lz")
        nc.scalar.activation(lz[:], zg[:], AF.Ln)
        rg = small.tile([128, 4], F32, tag="rg", name="rg")
        nc.scalar.activation(rg[:], lz[:], AF.Exp, scale=-0.5)
        onn = onp.tile([128, 4, 128], BF16, tag="onn", name="onn")
        for b in range(4):
            nc.vector.tensor_scalar(
                out=onn[:, b, :],
                in0=nat_sb[:, b, :],
                scalar1=rg[:, b : b + 1],
                scalar2=None,
                op0=OP.mult,
            )
        if DT_MODE in ("dma", "dmaq"):
            for b in range(4):
                nc.sync.dma_start(
                    outnT[h][:, ts(g * 4 + b, 128)], onn[:, b, :], transpose=True
                )
        else:
            tro = pvp.tile([128, 4, 128], BF16, tag="pv", name="tro")
            for b in range(4):
                nc.tensor.transpose(tro[:, b, :], onn[:, b, :], ident[:])
            nc.vector.tensor_copy(
                outnT[h][:, ts(g, 512)], tro[:].rearrange("p a b -> p (a b)")
            )

    def outproj_group(g):
        for tb in range(g * 4, g * 4 + 4):
            ya = pvp.tile([128, 512], F32, tag="pv", name="ya")
            for hh in range(EH):
                nc.tensor.matmul(
                    ya[:],
                    outnT[hh][:, ts(tb, 128)],
                    wo_t[hh][:, 0:512],
                    start=(hh == 0),
                    stop=(hh == EH - 1),
                )
            yb = pvp.tile([128, 512], F32, tag="pv", name="yb")
            for hh in range(EH):
                nc.tensor.matmul(
                    yb[:, 0:256],
                    outnT[hh][:, ts(tb, 128)],
                    wo_t[hh][:, 512:768],
                    start=(hh == 0),
                    stop=(hh == EH - 1),
                )
            yt = ysb.tile([128, D], F32, tag="y", name="yt")
            nc.vector.tensor_copy(yt[:, 0:512], ya[:])
            nc.vector.tensor_copy(yt[:, 512:768], yb[:, 0:256])
            reg_dma.dma_start(y[ts(tb, 128), :], yt[:])

    # ---- emission schedule ---------------------------------------------
    # Group tails are emitted one block late (software pipelining) so the
    # next group's score matmuls reach the PE queue ahead of PV/norm work.
    proj_qk(0)
    bg = proj_v_chunks() + proj_qk_chunks(1) + proj_qk_chunks(2)
    pending = None
    for h in range(EH):
        for g in range(4):
            dT = dtp.tile([128, NT, 512], BF16, tag="dT", name="dT")
            lamp_g = small.tile([128, 4], F32, tag="lamp", name="lamp")
            d1_g = small.tile([128, 4], F32, tag="d1g", name="d1g")
            for b in range(4):
                attn_block(h, g, b, dT, lamp_g, d1_g)
                if b == 0 and pending is not None:
                    pending()
                    pending = None
                for _ in range(2):
                    if bg:
                        bg.pop(0)()

            def tail(h=h, g=g, dT=dT, lamp_g=lamp_g, d1_g=d1_g):
                attn_group_tail(h, g, dT, lamp_g, d1_g)
                if h == EH - 1:
                    outproj_group(g)

            pending = tail
    pending()

def build_kernel():
    nc = bacc.Bacc("TRN2", target_bir_lowering=False, debug=False, num_devices=1)
    aps = [
        nc.dram_tensor("xT", [D, N], BF16, kind="ExternalInput").ap(),
        nc.dram_tensor("wq", [D, 384], BF16, kind="ExternalInput").ap(),
        nc.dram_tensor("wk", [D, 384], BF16, kind="ExternalInput").ap(),
        nc.dram_tensor("wv", [D, 384], BF16, kind="ExternalInput").ap(),
        nc.dram_tensor("wo", [384, D], BF16, kind="ExternalInput").ap(),
        nc.dram_tensor("bq", [3, 128], F32, kind="ExternalInput").ap(),
        nc.dram_tensor("bk", [3, 128], F32, kind="ExternalInput").ap(),
        nc.dram_tensor("bv128", [128, 384], F32, kind="ExternalInput").ap(),
        nc.dram_tensor("lam128", [128, 1], F32, kind="ExternalInput").ap(),
        nc.dram_tensor("y", [N, D], F32, kind="ExternalOutput").ap(),
    ]
    with tile.TileContext(nc, trace_sim=False) as tc:
        with ExitStack() as ctx:
            _body(ctx, tc, *aps)
    nc.compile()
    return nc


def make_in_maps(x, Wq, bqv, Wk, bkv, Wv, bvv, lambda_q1, lambda_k1,
                 lambda_q2, lambda_k2, norm_weight):
    scaling = HD ** -0.5
    lam1 = np.exp(np.sum(lambda_q1.astype(np.float64) * lambda_k1.astype(np.float64)))
    lam2 = np.exp(np.sum(lambda_q2.astype(np.float64) * lambda_k2.astype(np.float64)))
    lam = np.float32(lam1 - lam2 + LAMBDA_INIT)

    in_maps = []
    for c in range(NCORES):
        b = c // 2
        hs = 3 * (c % 2)
        cols = slice(128 * hs, 128 * (hs + 3))
        in_maps.append(
            {
                "xT": np.ascontiguousarray(x[b].T).astype(BF),
                "wq": np.ascontiguousarray(Wq[:, cols] * scaling).astype(BF),
                "wk": np.ascontiguousarray(Wk[:, cols]).astype(BF),
                "wv": np.ascontiguousarray(Wv[:, cols]).astype(BF),
                "wo": None,  # filled below (0.2*norm_weight folded in)
                "bq": (bqv[cols] * scaling).reshape(3, 128).astype(np.float32),
                "bk": bkv[cols].reshape(3, 128).astype(np.float32),
                "bv128": np.ascontiguousarray(
                    np.broadcast_to(bvv[cols], (128, 384))
                ).astype(np.float32),
                "lam128": np.full((128, 1), lam, np.float32),
            }
        )
    return in_maps, lam


_NC_CACHE = {}


def kernel(**inputs):
    x = np.asarray(inputs["x"], np.float32)
    Wq = np.asarray(inputs["Wq"], np.float32)
    bq = np.asarray(inputs["bq"], np.float32)
    Wk = np.asarray(inputs["Wk"], np.float32)
    bk = np.asarray(inputs["bk"], np.float32)
    Wv = np.asarray(inputs["Wv"], np.float32)
    bv = np.asarray(inputs["bv"], np.float32)
    Wo = np.asarray(inputs["Wo"], np.float32)
    bo = np.asarray(inputs["bo"], np.float32)
    norm_weight = np.asarray(inputs["norm_weight"], np.float32)

    in_maps, _lam = make_in_maps(
        x, Wq, bq, Wk, bk, Wv, bv,
        np.asarray(inputs["lambda_q1"], np.float32),
        np.asarray(inputs["lambda_k1"], np.float32),
        np.asarray(inputs["lambda_q2"], np.float32),
        np.asarray(inputs["lambda_k2"], np.float32),
        norm_weight,
    )
    # norm_weight is a per-(2*HD)-lane diagonal scale right before Wo:
    # out_n @ (diag(0.2*wn) @ Wo) == (out_n*0.2*wn) @ Wo.  Fold on host.
    wos = np.tile((1.0 - LAMBDA_INIT) * norm_weight, 6).reshape(768, 1) * Wo
    for c in range(NCORES):
        hs = 3 * (c % 2)
        cols = slice(128 * hs, 128 * (hs + 3))
        in_maps[c]["wo"] = np.ascontiguousarray(wos[cols, :]).astype(BF)

    if "nc" not in _NC_CACHE:
        _NC_CACHE["nc"] = build_kernel()
    nc = _NC_CACHE["nc"]

    res = bass_utils.run_bass_kernel_spmd(nc, in_maps, core_ids=list(range(NCORES)))

    out = np.empty((B, N, D), np.float32)
    for b in range(B):
        out[b] = res.results[2 * b]["y"] + res.results[2 * b + 1]["y"] + bo
    return out
